# revision 21
# baseline (speedup 1.0000x reference)
"""Trainium2 Bass kernel for nn_Head (single attention head over [B,T,S,E]).

Strategy: data-parallel over the 16 (b,t) pairs across 8 NeuronCores (2 each).
Per pair, on-device:
  q^/k^ projections computed in one merged matmul series (stationary
  [e, q|k] concat), written into zero-PADDED [128, S] buffers so that the
  score matmuls run with a full 128-row contraction (the PE array streams
  at ~2x the rate with all 8 row-groups loaded vs 4), v in [s, h] layout
  with an appended ones-column (softmax denominator for free during
  attn@v), scores computed TRANSPOSED St[k, q], exp on ScalarE directly
  from PSUM (scale=1/16 folded in), mask applied as a multiply by ~mask
  (host-transposed, bf16) on VectorE, augmented bf16 Wo projection
  (padded to 128-contraction) carrying the denominator and bo,
  PE-transpose of the bf16 result and per-partition normalization.
Masked entries become exactly 0 via the multiply; no max-subtraction needed
since scores/16 are O(1) and exp cannot overflow.
"""
import numpy as np
import ml_dtypes

import concourse.bass as bass
import concourse.tile as tile
from concourse import bacc, mybir
from concourse.bass_utils import run_bass_kernel_spmd
from concourse.masks import make_identity

B, T, S, E, H = 4, 4, 2048, 256, 64
NCORES = 8
PAIRS = B * T
PPC = PAIRS // NCORES  # pairs per core = 2
SLAB = 1024            # q-columns processed per PSUM slab
BF16 = mybir.dt.bfloat16
F32 = mybir.dt.float32
NPBF16 = ml_dtypes.bfloat16

_NC_CACHE = {}


def _build_nc(repeat=None):
    import os
    if repeat is None:
        repeat = int(os.environ.get("KREPEAT", "1"))
    nc = bacc.Bacc(None)
    xt_d = nc.declare_dram_parameter("xt", [PPC, E, S], BF16, isOutput=False)
    mt_d = nc.declare_dram_parameter("mt", [PPC, S, S], BF16, isOutput=False)
    wqk_d = nc.declare_dram_parameter("wqk", [2, 128, 128], BF16, isOutput=False)
    bqk_d = nc.declare_dram_parameter("bqk", [1, 128], BF16, isOutput=False)
    wv_d = nc.declare_dram_parameter("wv", [E, H], BF16, isOutput=False)
    bv_d = nc.declare_dram_parameter("bv", [1, H], BF16, isOutput=False)
    wo_d = nc.declare_dram_parameter("wo", [128, H + 1], BF16, isOutput=False)
    out_d = nc.declare_dram_parameter("out", [PPC, S, H], F32, isOutput=True)

    KT = S // 128          # 16 k-tiles per pair
    NSLAB = S // SLAB      # 2 q-slabs per pair
    SCH = S // 512         # 4 s-chunks for projections

    with tile.TileContext(nc) as tc:
        with (
            tc.tile_pool(name="singles", bufs=1) as singles,
            tc.tile_pool(name="xpool", bufs=4) as xpool,
            tc.tile_pool(name="vpool", bufs=4) as vpool,
            tc.tile_pool(name="mpool", bufs=16) as mpool,
            tc.tile_pool(name="epool", bufs=10) as epool,
            tc.tile_pool(name="apool", bufs=10) as apool,
            tc.tile_pool(name="ps_s", bufs=2, space="PSUM") as ps_s,
            tc.tile_pool(name="ps_o", bufs=1, space="PSUM") as ps_o,
            tc.tile_pool(name="ps_m", bufs=2, space="PSUM") as ps_m,
        ):
            # --- global constants ---
            wqk_sb = singles.tile([128, 2, 128], BF16, tag="wqk")
            for ec in range(2):
                nc.sync.dma_start(out=wqk_sb[:, ec, :], in_=wqk_d[ec, :, :])
            bqk_sb = singles.tile([1, 128], BF16, tag="bqk")
            nc.sync.dma_start(out=bqk_sb, in_=bqk_d[:])
            wv_sb = singles.tile([128, 2, H], BF16, tag="wv")
            for ec in range(2):
                nc.sync.dma_start(out=wv_sb[:, ec, :], in_=wv_d[ec * 128:(ec + 1) * 128, :])
            bv_sb = singles.tile([1, H], BF16, tag="bv")
            nc.sync.dma_start(out=bv_sb, in_=bv_d[:])
            wo_sb = singles.tile([128, H + 1], BF16, tag="wo")
            nc.sync.dma_start(out=wo_sb, in_=wo_d[:])
            ones_sb = singles.tile([1, S], BF16, tag="ones")
            nc.vector.memset(ones_sb, 1.0)
            identf = singles.tile([128, 128], F32, tag="identf")
            make_identity(nc, identf)
            ident = singles.tile([128, 128], BF16, tag="ident")
            nc.vector.tensor_copy(ident, identf)

            # persistent zero-padded q/k buffers (rows H..127 stay zero so the
            # score matmuls get a full 128-row contraction)
            q_bufs, k_bufs = [], []
            for p in range(PPC):
                qb = singles.tile([128, S], BF16, name=f"qb{p}", tag=f"qb{p}")
                kb = singles.tile([128, S], BF16, name=f"kb{p}", tag=f"kb{p}")
                nc.vector.memset(qb[H:128, :], 0.0)
                nc.vector.memset(kb[H:128, :], 0.0)
                q_bufs.append(qb)
                k_bufs.append(kb)
            # persistent zero-padded Wo-input and transpose-input buffers
            o_bufs = []
            for i in range(4):
                ob = singles.tile([128, SLAB], BF16, name=f"ob{i}", tag=f"ob{i}")
                nc.vector.memset(ob[H:128, :], 0.0)
                o_bufs.append(ob)
            pt_bufs = []
            for i in range(2):
                pb = singles.tile([128, 512], BF16, name=f"ptb{i}", tag=f"ptb{i}")
                nc.vector.memset(pb[H:128, :], 0.0)
                pt_bufs.append(pb)

            for _rep in range(repeat):
             deferred = []
             for p in range(PPC):
                 # ---------------- phase A: projections ----------------
                 xt_sb = xpool.tile([128, 2, S], BF16, tag="xt")
                 for ec in range(2):
                     nc.sync.dma_start(out=xt_sb[:, ec, :], in_=xt_d[p, ec * 128:(ec + 1) * 128, :])

                 q_sb, k_sb = q_bufs[p], k_bufs[p]
                 for sc in range(SCH):
                     pr = ps_m.tile([128, 512], F32, tag="pm")
                     csl = slice(sc * 512, (sc + 1) * 512)
                     nc.tensor.matmul(pr, wqk_sb[:, 0, :], xt_sb[:, 0, csl],
                                      start=True, stop=False)
                     nc.tensor.matmul(pr, wqk_sb[:, 1, :], xt_sb[:, 1, csl],
                                      start=False, stop=False)
                     nc.tensor.matmul(pr, bqk_sb, ones_sb[:, csl], start=False, stop=True)
                     nc.vector.tensor_copy(q_sb[0:H, csl], pr[0:H, :])
                     nc.vector.tensor_copy(k_sb[0:H, csl], pr[H:128, :])

                 # v in [s, h] layout + ones column (denominator trick)
                 v_sb = vpool.tile([128, KT, H + 1], BF16, tag="v")
                 nc.vector.memset(v_sb[:, :, H:H + 1], 1.0)
                 for st in range(KT):
                     pv = ps_m.tile([128, 512], F32, tag="pm")
                     ssl = slice(st * 128, (st + 1) * 128)
                     nc.tensor.matmul(pv[:, 0:H], xt_sb[:, 0, ssl], wv_sb[:, 0, :],
                                      start=True, stop=False)
                     nc.tensor.matmul(pv[:, 0:H], xt_sb[:, 1, ssl], wv_sb[:, 1, :],
                                      start=False, stop=False)
                     nc.tensor.matmul(pv[:, 0:H], ones_sb[:, ssl], bv_sb,
                                      start=False, stop=True)
                     nc.vector.tensor_copy(v_sb[:, st, 0:H], pv[:, 0:H])

                 # ---------------- phase B: attention ----------------
                 for sl in range(NSLAB):
                     out_ps = ps_o.tile([H + 1, SLAB], F32, tag="po")
                     q0 = sl * SLAB
                     for kt in range(KT):
                         s_ps = ps_s.tile([128, SLAB], F32, tag="ps")
                         ksl = slice(kt * 128, (kt + 1) * 128)
                         for hf in range(SLAB // 512):
                             nc.tensor.matmul(
                                 s_ps[:, hf * 512:(hf + 1) * 512],
                                 k_sb[:, ksl],
                                 q_sb[:, q0 + hf * 512:q0 + (hf + 1) * 512],
                                 start=True, stop=True)
                         m_sb = mpool.tile([128, SLAB], BF16, tag="m")
                         nc.gpsimd.dma_start(out=m_sb, in_=mt_d[p, ksl, q0:q0 + SLAB])
                         e_sb = epool.tile([128, SLAB], BF16, tag="e")
                         nc.scalar.activation(out=e_sb, in_=s_ps,
                                              func=mybir.ActivationFunctionType.Exp,
                                              scale=1.0 / 16.0)
                         at_sb = apool.tile([128, SLAB], BF16, tag="at")
                         nc.vector.tensor_mul(at_sb, e_sb, m_sb)
                         for hf in range(SLAB // 512):
                             nc.tensor.matmul(
                                 out_ps[:, hf * 512:(hf + 1) * 512],
                                 v_sb[:, kt, :],
                                 at_sb[:, hf * 512:(hf + 1) * 512],
                                 start=(kt == 0), stop=(kt == KT - 1))

                     # stash numerator+denominator; epilogue deferred to end
                     o_sb = o_bufs[(p * NSLAB + sl) % 4]
                     nc.vector.tensor_copy(o_sb[0:H + 1, :], out_ps)
                     deferred.append((p, q0, o_sb))

             # ------- deferred epilogues: Wo + transpose + normalize -------
             for di, (p, q0, o_sb) in enumerate(deferred):
                 for nch in range(SLAB // 512):
                     pp = ps_m.tile([128, 512], F32, tag="pm")
                     nc.tensor.matmul(pp[0:H + 1, :], wo_sb,
                                      o_sb[:, nch * 512:(nch + 1) * 512],
                                      start=True, stop=True)
                     pt_sb = pt_bufs[nch % 2]
                     nc.vector.tensor_copy(pt_sb[0:H + 1, :], pp[0:H + 1, :])
                     tr = ps_m.tile([128, 512], BF16, tag="pm")
                     for i in range(4):
                         nc.tensor.transpose(tr[:, i * 128:(i + 1) * 128],
                                             pt_sb[:, i * 128:(i + 1) * 128],
                                             ident)
                     r_sb = epool.tile([128, 4], F32, tag="r")
                     nc.vector.reciprocal(r_sb, tr[:, H::128])
                     for i in range(4):
                         of_sb = apool.tile([128, H], F32, name=f"of", tag="of")
                         nc.vector.tensor_scalar_mul(
                             of_sb, tr[:, i * 128:i * 128 + H], r_sb[:, i:i + 1])
                         row0 = q0 + nch * 512 + i * 128
                         nc.sync.dma_start(out=out_d[p, row0:row0 + 128, :], in_=of_sb)

    nc.compile()
    return nc


def get_nc():
    if "nc" not in _NC_CACHE:
        _NC_CACHE["nc"] = _build_nc()
    return _NC_CACHE["nc"]


def _host_prep(x, mask, Wq, bq, Wk, bk, Wv, bv, Wo, bo):
    x = np.asarray(x, dtype=np.float32)
    mask = np.asarray(mask)
    xt = np.ascontiguousarray(x.transpose(0, 1, 3, 2)).astype(NPBF16)      # [B,T,E,S]
    mt = np.ascontiguousarray(
        (~mask).transpose(0, 1, 3, 2)).astype(NPBF16)                      # [B,T,S(k),S(q)]
    wqT = np.asarray(Wq, np.float32).T                                     # [E,H]
    wkT = np.asarray(Wk, np.float32).T
    wqk = np.stack([
        np.concatenate([wqT[ec * 128:(ec + 1) * 128], wkT[ec * 128:(ec + 1) * 128]],
                       axis=1)
        for ec in range(2)]).astype(NPBF16)                                # [2,128,128]
    bqk = np.concatenate([np.asarray(bq, np.float32), np.asarray(bk, np.float32)]
                         ).reshape(1, 128).astype(NPBF16)
    wv = np.ascontiguousarray(np.asarray(Wv, np.float32).T).astype(NPBF16)  # [E,H]
    bvr = np.asarray(bv, np.float32).reshape(1, H).astype(NPBF16)
    wo_pad = np.zeros((128, H + 1), np.float32)
    wo_pad[:H, :H] = np.asarray(Wo, np.float32).T
    wo_pad[H, :H] = np.asarray(bo, np.float32)
    wo_pad[H, H] = 1.0
    wo_pad = wo_pad.astype(NPBF16)
    xt = xt.reshape(PAIRS, E, S)
    mt = mt.reshape(PAIRS, S, S)
    return xt, mt, wqk, bqk, wv, bvr, wo_pad


def make_in_maps(x, mask, Wq, bq, Wk, bk, Wv, bv, Wo, bo):
    xt, mt, wqk, bqk, wv, bvr, wo_pad = _host_prep(
        x, mask, Wq, bq, Wk, bk, Wv, bv, Wo, bo)
    in_maps = []
    for c in range(NCORES):
        sl = slice(c * PPC, (c + 1) * PPC)
        in_maps.append({
            "xt": np.ascontiguousarray(xt[sl]),
            "mt": np.ascontiguousarray(mt[sl]),
            "wqk": wqk, "bqk": bqk, "wv": wv, "bv": bvr,
            "wo": wo_pad,
        })
    return in_maps


def kernel(x, mask, Wq, bq, Wk, bk, Wv, bv, Wo, bo, **kw):
    nc = get_nc()
    in_maps = make_in_maps(x, mask, Wq, bq, Wk, bk, Wv, bv, Wo, bo)
    res = run_bass_kernel_spmd(nc, in_maps, core_ids=list(range(NCORES)))
    outs = [res.results[c]["out"] for c in range(NCORES)]
    full = np.concatenate(outs, axis=0)          # [PAIRS, S, H]
    return np.ascontiguousarray(full.reshape(B, T, S, H).astype(np.float32))


# revision 22
# speedup vs baseline: 1.0225x; 1.0225x over previous
"""Trainium2 Bass kernel for nn_Head (single attention head over [B,T,S,E]).

Strategy: data-parallel over the 16 (b,t) pairs across 8 NeuronCores (2 each).
Per pair, on-device:
  q^/k^ projections computed in one merged matmul series (stationary
  [e, q|k] concat), written into zero-PADDED [128, S] buffers so that the
  score matmuls run with a full 128-row contraction (the PE array streams
  at ~2x the rate with all 8 row-groups loaded vs 4), v in [s, h] layout
  with an appended ones-column (softmax denominator for free during
  attn@v), scores computed TRANSPOSED St[k, q], exp on ScalarE directly
  from PSUM (scale=1/16 folded in), mask applied as a multiply by ~mask
  (host-transposed, bf16) on VectorE, augmented bf16 Wo projection
  (padded to 128-contraction) carrying the denominator and bo,
  PE-transpose of the bf16 result and per-partition normalization.
Masked entries become exactly 0 via the multiply; no max-subtraction needed
since scores/16 are O(1) and exp cannot overflow.
"""
import numpy as np
import ml_dtypes

import concourse.bass as bass
import concourse.tile as tile
from concourse import bacc, mybir
from concourse.bass_utils import run_bass_kernel_spmd
from concourse.masks import make_identity

B, T, S, E, H = 4, 4, 2048, 256, 64
NCORES = 8
PAIRS = B * T
PPC = PAIRS // NCORES  # pairs per core = 2
SLAB = 1024            # q-columns processed per PSUM slab
BF16 = mybir.dt.bfloat16
F32 = mybir.dt.float32
NPBF16 = ml_dtypes.bfloat16

_NC_CACHE = {}


def _build_nc(repeat=None):
    import os
    if repeat is None:
        repeat = int(os.environ.get("KREPEAT", "1"))
    nc = bacc.Bacc(None)
    xt_d = nc.declare_dram_parameter("xt", [PPC, E, S], BF16, isOutput=False)
    mt_d = nc.declare_dram_parameter("mt", [PPC, S, S], BF16, isOutput=False)
    wqk_d = nc.declare_dram_parameter("wqk", [2, 128, 128], BF16, isOutput=False)
    bqk_d = nc.declare_dram_parameter("bqk", [1, 128], BF16, isOutput=False)
    wv_d = nc.declare_dram_parameter("wv", [E, H], BF16, isOutput=False)
    bv_d = nc.declare_dram_parameter("bv", [1, H], BF16, isOutput=False)
    wo_d = nc.declare_dram_parameter("wo", [128, H + 1], BF16, isOutput=False)
    out_d = nc.declare_dram_parameter("out", [PPC, S, H], F32, isOutput=True)

    KT = S // 128          # 16 k-tiles per pair
    NSLAB = S // SLAB      # 2 q-slabs per pair
    SCH = S // 512         # 4 s-chunks for projections

    with tile.TileContext(nc) as tc:
        with (
            tc.tile_pool(name="singles", bufs=1) as singles,
            tc.tile_pool(name="xpool", bufs=3) as xpool,
            tc.tile_pool(name="vpool", bufs=3) as vpool,
            tc.tile_pool(name="mpool", bufs=12) as mpool,
            tc.tile_pool(name="epool", bufs=8) as epool,
            tc.tile_pool(name="apool", bufs=8) as apool,
            tc.tile_pool(name="ps_s", bufs=2, space="PSUM") as ps_s,
            tc.tile_pool(name="ps_o", bufs=1, space="PSUM") as ps_o,
            tc.tile_pool(name="ps_m", bufs=2, space="PSUM") as ps_m,
        ):
            # --- global constants ---
            wqk_sb = singles.tile([128, 2, 128], BF16, tag="wqk")
            for ec in range(2):
                nc.sync.dma_start(out=wqk_sb[:, ec, :], in_=wqk_d[ec, :, :])
            bqk_sb = singles.tile([1, 128], BF16, tag="bqk")
            nc.sync.dma_start(out=bqk_sb, in_=bqk_d[:])
            wv_sb = singles.tile([128, 2, H], BF16, tag="wv")
            for ec in range(2):
                nc.sync.dma_start(out=wv_sb[:, ec, :], in_=wv_d[ec * 128:(ec + 1) * 128, :])
            bv_sb = singles.tile([1, H], BF16, tag="bv")
            nc.sync.dma_start(out=bv_sb, in_=bv_d[:])
            wo_sb = singles.tile([128, H + 1], BF16, tag="wo")
            nc.sync.dma_start(out=wo_sb, in_=wo_d[:])
            ones_sb = singles.tile([1, S], BF16, tag="ones")
            nc.vector.memset(ones_sb, 1.0)
            identf = singles.tile([128, 128], F32, tag="identf")
            make_identity(nc, identf)
            ident = singles.tile([128, 128], BF16, tag="ident")
            nc.vector.tensor_copy(ident, identf)

            # persistent zero-padded q/k buffers (rows H..127 stay zero so the
            # score matmuls get a full 128-row contraction)
            q_bufs, k_bufs = [], []
            for p in range(PPC):
                qb = singles.tile([128, S], BF16, name=f"qb{p}", tag=f"qb{p}")
                kb = singles.tile([128, S], BF16, name=f"kb{p}", tag=f"kb{p}")
                nc.vector.memset(qb[H:128, :], 0.0)
                nc.vector.memset(kb[H:128, :], 0.0)
                q_bufs.append(qb)
                k_bufs.append(kb)
            # persistent zero-padded Wo-input and transpose-input buffers
            o_bufs = []
            for i in range(4):
                ob = singles.tile([128, SLAB], BF16, name=f"ob{i}", tag=f"ob{i}")
                nc.vector.memset(ob[H:128, :], 0.0)
                o_bufs.append(ob)
            pt_bufs = []
            for i in range(2):
                pb = singles.tile([128, 512], BF16, name=f"ptb{i}", tag=f"ptb{i}")
                nc.vector.memset(pb[H:128, :], 0.0)
                pt_bufs.append(pb)

            for _rep in range(repeat):
             deferred = []
             for p in range(PPC):
                 # ---------------- phase A: projections ----------------
                 xt_sb = xpool.tile([128, 2, S], BF16, tag="xt")
                 for ec in range(2):
                     nc.sync.dma_start(out=xt_sb[:, ec, :], in_=xt_d[p, ec * 128:(ec + 1) * 128, :])

                 q_sb, k_sb = q_bufs[p], k_bufs[p]
                 for sc in range(SCH):
                     pr = ps_m.tile([128, 512], F32, tag="pm")
                     csl = slice(sc * 512, (sc + 1) * 512)
                     nc.tensor.matmul(pr, wqk_sb[:, 0, :], xt_sb[:, 0, csl],
                                      start=True, stop=False)
                     nc.tensor.matmul(pr, wqk_sb[:, 1, :], xt_sb[:, 1, csl],
                                      start=False, stop=False)
                     nc.tensor.matmul(pr, bqk_sb, ones_sb[:, csl], start=False, stop=True)
                     nc.vector.tensor_copy(q_sb[0:H, csl], pr[0:H, :])
                     nc.vector.tensor_copy(k_sb[0:H, csl], pr[H:128, :])

                 # v in [s, h] layout + ones column (denominator trick)
                 v_sb = vpool.tile([128, KT, H + 1], BF16, tag="v")
                 nc.vector.memset(v_sb[:, :, H:H + 1], 1.0)
                 for st in range(KT):
                     pv = ps_m.tile([128, 512], F32, tag="pm")
                     ssl = slice(st * 128, (st + 1) * 128)
                     nc.tensor.matmul(pv[:, 0:H], xt_sb[:, 0, ssl], wv_sb[:, 0, :],
                                      start=True, stop=False)
                     nc.tensor.matmul(pv[:, 0:H], xt_sb[:, 1, ssl], wv_sb[:, 1, :],
                                      start=False, stop=False)
                     nc.tensor.matmul(pv[:, 0:H], ones_sb[:, ssl], bv_sb,
                                      start=False, stop=True)
                     nc.vector.tensor_copy(v_sb[:, st, 0:H], pv[:, 0:H])

                 # ---------------- phase B: attention ----------------
                 for sl in range(NSLAB):
                     out_ps = ps_o.tile([H + 1, SLAB], F32, tag="po")
                     q0 = sl * SLAB
                     for kt in range(KT):
                         s_ps = ps_s.tile([128, SLAB], F32, tag="ps")
                         ksl = slice(kt * 128, (kt + 1) * 128)
                         for hf in range(SLAB // 512):
                             nc.tensor.matmul(
                                 s_ps[:, hf * 512:(hf + 1) * 512],
                                 k_sb[:, ksl],
                                 q_sb[:, q0 + hf * 512:q0 + (hf + 1) * 512],
                                 start=True, stop=True)
                         m_sb = mpool.tile([128, SLAB], BF16, tag="m")
                         nc.gpsimd.dma_start(out=m_sb, in_=mt_d[p, ksl, q0:q0 + SLAB])
                         e_sb = epool.tile([128, SLAB], BF16, tag="e")
                         nc.scalar.activation(out=e_sb, in_=s_ps,
                                              func=mybir.ActivationFunctionType.Exp,
                                              scale=1.0 / 16.0)
                         at_sb = apool.tile([128, SLAB], BF16, tag="at")
                         nc.vector.tensor_mul(at_sb, e_sb, m_sb)
                         for hf in range(SLAB // 512):
                             nc.tensor.matmul(
                                 out_ps[:, hf * 512:(hf + 1) * 512],
                                 v_sb[:, kt, :],
                                 at_sb[:, hf * 512:(hf + 1) * 512],
                                 start=(kt == 0), stop=(kt == KT - 1))

                     # stash numerator+denominator; epilogue deferred to end
                     o_sb = o_bufs[(p * NSLAB + sl) % 4]
                     nc.vector.tensor_copy(o_sb[0:H + 1, :], out_ps)
                     deferred.append((p, q0, o_sb))

             # ------- deferred epilogues: Wo + transpose + normalize -------
             for di, (p, q0, o_sb) in enumerate(deferred):
                 for nch in range(SLAB // 512):
                     pp = ps_m.tile([128, 512], F32, tag="pm")
                     nc.tensor.matmul(pp[0:H + 1, :], wo_sb,
                                      o_sb[:, nch * 512:(nch + 1) * 512],
                                      start=True, stop=True)
                     pt_sb = pt_bufs[nch % 2]
                     nc.vector.tensor_copy(pt_sb[0:H + 1, :], pp[0:H + 1, :])
                     tr = ps_m.tile([128, 512], BF16, tag="pm")
                     for i in range(4):
                         nc.tensor.transpose(tr[:, i * 128:(i + 1) * 128],
                                             pt_sb[:, i * 128:(i + 1) * 128],
                                             ident)
                     r_sb = epool.tile([128, 4], F32, tag="r")
                     nc.vector.reciprocal(r_sb, tr[:, H::128])
                     for i in range(4):
                         of_sb = apool.tile([128, H], F32, name=f"of", tag="of")
                         nc.vector.tensor_scalar_mul(
                             of_sb, tr[:, i * 128:i * 128 + H], r_sb[:, i:i + 1])
                         row0 = q0 + nch * 512 + i * 128
                         nc.sync.dma_start(out=out_d[p, row0:row0 + 128, :], in_=of_sb)

    nc.compile()
    return nc


def get_nc():
    if "nc" not in _NC_CACHE:
        _NC_CACHE["nc"] = _build_nc()
    return _NC_CACHE["nc"]


def _host_prep(x, mask, Wq, bq, Wk, bk, Wv, bv, Wo, bo):
    x = np.asarray(x, dtype=np.float32)
    mask = np.asarray(mask)
    xt = np.ascontiguousarray(x.transpose(0, 1, 3, 2)).astype(NPBF16)      # [B,T,E,S]
    mt = np.ascontiguousarray(
        (~mask).transpose(0, 1, 3, 2)).astype(NPBF16)                      # [B,T,S(k),S(q)]
    wqT = np.asarray(Wq, np.float32).T                                     # [E,H]
    wkT = np.asarray(Wk, np.float32).T
    wqk = np.stack([
        np.concatenate([wqT[ec * 128:(ec + 1) * 128], wkT[ec * 128:(ec + 1) * 128]],
                       axis=1)
        for ec in range(2)]).astype(NPBF16)                                # [2,128,128]
    bqk = np.concatenate([np.asarray(bq, np.float32), np.asarray(bk, np.float32)]
                         ).reshape(1, 128).astype(NPBF16)
    wv = np.ascontiguousarray(np.asarray(Wv, np.float32).T).astype(NPBF16)  # [E,H]
    bvr = np.asarray(bv, np.float32).reshape(1, H).astype(NPBF16)
    wo_pad = np.zeros((128, H + 1), np.float32)
    wo_pad[:H, :H] = np.asarray(Wo, np.float32).T
    wo_pad[H, :H] = np.asarray(bo, np.float32)
    wo_pad[H, H] = 1.0
    wo_pad = wo_pad.astype(NPBF16)
    xt = xt.reshape(PAIRS, E, S)
    mt = mt.reshape(PAIRS, S, S)
    return xt, mt, wqk, bqk, wv, bvr, wo_pad


def make_in_maps(x, mask, Wq, bq, Wk, bk, Wv, bv, Wo, bo):
    xt, mt, wqk, bqk, wv, bvr, wo_pad = _host_prep(
        x, mask, Wq, bq, Wk, bk, Wv, bv, Wo, bo)
    in_maps = []
    for c in range(NCORES):
        sl = slice(c * PPC, (c + 1) * PPC)
        in_maps.append({
            "xt": np.ascontiguousarray(xt[sl]),
            "mt": np.ascontiguousarray(mt[sl]),
            "wqk": wqk, "bqk": bqk, "wv": wv, "bv": bvr,
            "wo": wo_pad,
        })
    return in_maps


def kernel(x, mask, Wq, bq, Wk, bk, Wv, bv, Wo, bo, **kw):
    nc = get_nc()
    in_maps = make_in_maps(x, mask, Wq, bq, Wk, bk, Wv, bv, Wo, bo)
    res = run_bass_kernel_spmd(nc, in_maps, core_ids=list(range(NCORES)))
    outs = [res.results[c]["out"] for c in range(NCORES)]
    full = np.concatenate(outs, axis=0)          # [PAIRS, S, H]
    return np.ascontiguousarray(full.reshape(B, T, S, H).astype(np.float32))


# revision 24
# speedup vs baseline: 1.1725x; 1.1467x over previous
"""Trainium2 Bass kernel for nn_Head (single attention head over [B,T,S,E]).

Strategy: data-parallel over the 16 (b,t) pairs across 8 NeuronCores (2 each).
Per pair, on-device:
  q^/k^ projections computed in one merged matmul series (stationary
  [e, q|k] concat), written into zero-PADDED [128, S] buffers so that the
  score matmuls run with a full 128-row contraction (the PE array streams
  at ~2x the rate with all 8 row-groups loaded vs 4), v in [s, h] layout
  with an appended ones-column (softmax denominator for free during
  attn@v), scores computed TRANSPOSED St[k, q], exp on ScalarE directly
  from PSUM (scale=1/16 folded in), mask applied as a multiply by ~mask
  (host-transposed, bf16) on VectorE, augmented bf16 Wo projection
  (padded to 128-contraction) carrying the denominator and bo,
  PE-transpose of the bf16 result and per-partition normalization.
Masked entries become exactly 0 via the multiply; no max-subtraction needed
since scores/16 are O(1) and exp cannot overflow.
"""
import numpy as np
import ml_dtypes

import concourse.bass as bass
import concourse.tile as tile
from concourse import bacc, mybir
from concourse.bass_utils import run_bass_kernel_spmd
from concourse.masks import make_identity

B, T, S, E, H = 4, 4, 2048, 256, 64
NCORES = 8
PAIRS = B * T
PPC = PAIRS // NCORES  # pairs per core = 2
SLAB = 1024            # q-columns processed per PSUM slab
BF16 = mybir.dt.bfloat16
F32 = mybir.dt.float32
NPBF16 = ml_dtypes.bfloat16

_NC_CACHE = {}


def _build_nc(repeat=None):
    import os
    if repeat is None:
        repeat = int(os.environ.get("KREPEAT", "1"))
    nc = bacc.Bacc(None)
    xt_d = nc.declare_dram_parameter("xt", [PPC, E, S], BF16, isOutput=False)
    mt_d = nc.declare_dram_parameter("mt", [PPC, S, S], BF16, isOutput=False)
    wqk_d = nc.declare_dram_parameter("wqk", [2, 128, 128], BF16, isOutput=False)
    bqk_d = nc.declare_dram_parameter("bqk", [128, 1], F32, isOutput=False)
    wv_d = nc.declare_dram_parameter("wv", [E, H], BF16, isOutput=False)
    wo_d = nc.declare_dram_parameter("wo", [128, H + 1], BF16, isOutput=False)
    out_d = nc.declare_dram_parameter("out", [PPC, S, H], F32, isOutput=True)

    KT = S // 128          # 16 k-tiles per pair
    NSLAB = S // SLAB      # 2 q-slabs per pair
    SCH = S // 512         # 4 s-chunks for projections

    with tile.TileContext(nc) as tc:
        with (
            tc.tile_pool(name="singles", bufs=1) as singles,
            tc.tile_pool(name="xpool", bufs=3) as xpool,
            tc.tile_pool(name="vpool", bufs=3) as vpool,
            tc.tile_pool(name="mpool", bufs=12) as mpool,
            tc.tile_pool(name="epool", bufs=8) as epool,
            tc.tile_pool(name="apool", bufs=8) as apool,
            tc.tile_pool(name="ps_s", bufs=2, space="PSUM") as ps_s,
            tc.tile_pool(name="ps_o", bufs=1, space="PSUM") as ps_o,
            tc.tile_pool(name="ps_m", bufs=2, space="PSUM") as ps_m,
        ):
            # --- global constants ---
            wqk_sb = singles.tile([128, 2, 128], BF16, tag="wqk")
            for ec in range(2):
                nc.sync.dma_start(out=wqk_sb[:, ec, :], in_=wqk_d[ec, :, :])
            bqk_sb = singles.tile([128, 1], F32, tag="bqk")
            nc.sync.dma_start(out=bqk_sb, in_=bqk_d[:])
            wv_sb = singles.tile([128, 2, H], BF16, tag="wv")
            for ec in range(2):
                nc.sync.dma_start(out=wv_sb[:, ec, :], in_=wv_d[ec * 128:(ec + 1) * 128, :])
            wo_sb = singles.tile([128, H + 1], BF16, tag="wo")
            nc.sync.dma_start(out=wo_sb, in_=wo_d[:])
            identf = singles.tile([128, 128], F32, tag="identf")
            make_identity(nc, identf)
            ident = singles.tile([128, 128], BF16, tag="ident")
            nc.vector.tensor_copy(ident, identf)

            # persistent zero-padded q/k buffers (rows H..127 stay zero so the
            # score matmuls get a full 128-row contraction)
            q_bufs, k_bufs = [], []
            for p in range(PPC):
                qb = singles.tile([128, S], BF16, name=f"qb{p}", tag=f"qb{p}")
                kb = singles.tile([128, S], BF16, name=f"kb{p}", tag=f"kb{p}")
                nc.vector.memset(qb[H:128, :], 0.0)
                nc.vector.memset(kb[H:128, :], 0.0)
                q_bufs.append(qb)
                k_bufs.append(kb)
            # persistent zero-padded Wo-input and transpose-input buffers
            o_bufs = []
            for i in range(4):
                ob = singles.tile([128, SLAB], BF16, name=f"ob{i}", tag=f"ob{i}")
                nc.vector.memset(ob[H:128, :], 0.0)
                o_bufs.append(ob)
            pt_bufs = []
            for i in range(2):
                pb = singles.tile([128, 512], BF16, name=f"ptb{i}", tag=f"ptb{i}")
                nc.vector.memset(pb[H:128, :], 0.0)
                pt_bufs.append(pb)

            for _rep in range(repeat):
             deferred = []
             for p in range(PPC):
                 # ---------------- phase A: projections ----------------
                 xt_sb = xpool.tile([128, 2, S], BF16, tag="xt")
                 for ec in range(2):
                     nc.sync.dma_start(out=xt_sb[:, ec, :], in_=xt_d[p, ec * 128:(ec + 1) * 128, :])

                 q_sb, k_sb = q_bufs[p], k_bufs[p]
                 for sc in range(SCH):
                     pr = ps_m.tile([128, 512], F32, tag="pm")
                     csl = slice(sc * 512, (sc + 1) * 512)
                     nc.tensor.matmul(pr, wqk_sb[:, 0, :], xt_sb[:, 0, csl],
                                      start=True, stop=False)
                     nc.tensor.matmul(pr, wqk_sb[:, 1, :], xt_sb[:, 1, csl],
                                      start=False, stop=True)
                     # bias folded into the PSUM->SBUF copy (per-partition add)
                     nc.vector.tensor_scalar_add(q_sb[0:H, csl], pr[0:H, :],
                                                 bqk_sb[0:H, :])
                     nc.vector.tensor_scalar_add(k_sb[0:H, csl], pr[H:128, :],
                                                 bqk_sb[H:128, :])

                 # v in [s, h] layout + ones column (denominator trick)
                 v_sb = vpool.tile([128, KT, H + 1], BF16, tag="v")
                 nc.vector.memset(v_sb[:, :, H:H + 1], 1.0)
                 for st in range(KT):
                     pv = ps_m.tile([128, 512], F32, tag="pm")
                     ssl = slice(st * 128, (st + 1) * 128)
                     nc.tensor.matmul(pv[:, 0:H], xt_sb[:, 0, ssl], wv_sb[:, 0, :],
                                      start=True, stop=False)
                     nc.tensor.matmul(pv[:, 0:H], xt_sb[:, 1, ssl], wv_sb[:, 1, :],
                                      start=False, stop=True)
                     nc.vector.tensor_copy(v_sb[:, st, 0:H], pv[:, 0:H])

                 # ---------------- phase B: attention ----------------
                 for sl in range(NSLAB):
                     out_ps = ps_o.tile([H + 1, SLAB], F32, tag="po")
                     q0 = sl * SLAB
                     for kt in range(KT):
                         s_ps = ps_s.tile([128, SLAB], F32, tag="ps")
                         ksl = slice(kt * 128, (kt + 1) * 128)
                         for hf in range(SLAB // 512):
                             nc.tensor.matmul(
                                 s_ps[:, hf * 512:(hf + 1) * 512],
                                 k_sb[:, ksl],
                                 q_sb[:, q0 + hf * 512:q0 + (hf + 1) * 512],
                                 start=True, stop=True)
                         m_sb = mpool.tile([128, SLAB], BF16, tag="m")
                         nc.gpsimd.dma_start(out=m_sb, in_=mt_d[p, ksl, q0:q0 + SLAB])
                         e_sb = epool.tile([128, SLAB], BF16, tag="e")
                         nc.scalar.activation(out=e_sb, in_=s_ps,
                                              func=mybir.ActivationFunctionType.Exp,
                                              scale=1.0 / 16.0)
                         at_sb = apool.tile([128, SLAB], BF16, tag="at")
                         nc.vector.tensor_mul(at_sb, e_sb, m_sb)
                         for hf in range(SLAB // 512):
                             nc.tensor.matmul(
                                 out_ps[:, hf * 512:(hf + 1) * 512],
                                 v_sb[:, kt, :],
                                 at_sb[:, hf * 512:(hf + 1) * 512],
                                 start=(kt == 0), stop=(kt == KT - 1))

                     # stash numerator+denominator; epilogue deferred to end
                     o_sb = o_bufs[(p * NSLAB + sl) % 4]
                     nc.vector.tensor_copy(o_sb[0:H + 1, :], out_ps)
                     deferred.append((p, q0, o_sb))

             # ------- deferred epilogues: Wo + transpose + normalize -------
             for di, (p, q0, o_sb) in enumerate(deferred):
                 for nch in range(SLAB // 512):
                     pp = ps_m.tile([128, 512], F32, tag="pm")
                     nc.tensor.matmul(pp[0:H + 1, :], wo_sb,
                                      o_sb[:, nch * 512:(nch + 1) * 512],
                                      start=True, stop=True)
                     pt_sb = pt_bufs[nch % 2]
                     nc.vector.tensor_copy(pt_sb[0:H + 1, :], pp[0:H + 1, :])
                     tr = ps_m.tile([128, 512], BF16, tag="pm")
                     for i in range(4):
                         nc.tensor.transpose(tr[:, i * 128:(i + 1) * 128],
                                             pt_sb[:, i * 128:(i + 1) * 128],
                                             ident)
                     r_sb = epool.tile([128, 4], F32, tag="r")
                     nc.vector.reciprocal(r_sb, tr[:, H::128])
                     for i in range(4):
                         of_sb = apool.tile([128, H], F32, name=f"of", tag="of")
                         nc.vector.tensor_scalar_mul(
                             of_sb, tr[:, i * 128:i * 128 + H], r_sb[:, i:i + 1])
                         row0 = q0 + nch * 512 + i * 128
                         nc.sync.dma_start(out=out_d[p, row0:row0 + 128, :], in_=of_sb)

    nc.compile()
    return nc


def get_nc():
    if "nc" not in _NC_CACHE:
        _NC_CACHE["nc"] = _build_nc()
    return _NC_CACHE["nc"]


def _host_prep(x, mask, Wq, bq, Wk, bk, Wv, bv, Wo, bo):
    x = np.asarray(x, dtype=np.float32)
    mask = np.asarray(mask)
    xt = np.ascontiguousarray(x.transpose(0, 1, 3, 2)).astype(NPBF16)      # [B,T,E,S]
    mt = np.ascontiguousarray(
        (~mask).transpose(0, 1, 3, 2)).astype(NPBF16)                      # [B,T,S(k),S(q)]
    wqT = np.asarray(Wq, np.float32).T                                     # [E,H]
    wkT = np.asarray(Wk, np.float32).T
    wqk = np.stack([
        np.concatenate([wqT[ec * 128:(ec + 1) * 128], wkT[ec * 128:(ec + 1) * 128]],
                       axis=1)
        for ec in range(2)]).astype(NPBF16)                                # [2,128,128]
    bqk = np.concatenate([np.asarray(bq, np.float32), np.asarray(bk, np.float32)]
                         ).reshape(128, 1)
    wv = np.ascontiguousarray(np.asarray(Wv, np.float32).T).astype(NPBF16)  # [E,H]
    wo_pad = np.zeros((128, H + 1), np.float32)
    wo_pad[:H, :H] = np.asarray(Wo, np.float32).T
    # v-bias folded here: attn(v + bv) = attn(v) + bv, so the denominator
    # row carries (Wo @ bv + bo) instead of bo
    wo_pad[H, :H] = (np.asarray(Wo, np.float32) @ np.asarray(bv, np.float32)
                     + np.asarray(bo, np.float32))
    wo_pad[H, H] = 1.0
    wo_pad = wo_pad.astype(NPBF16)
    xt = xt.reshape(PAIRS, E, S)
    mt = mt.reshape(PAIRS, S, S)
    return xt, mt, wqk, bqk, wv, wo_pad


def make_in_maps(x, mask, Wq, bq, Wk, bk, Wv, bv, Wo, bo):
    xt, mt, wqk, bqk, wv, wo_pad = _host_prep(
        x, mask, Wq, bq, Wk, bk, Wv, bv, Wo, bo)
    in_maps = []
    for c in range(NCORES):
        sl = slice(c * PPC, (c + 1) * PPC)
        in_maps.append({
            "xt": np.ascontiguousarray(xt[sl]),
            "mt": np.ascontiguousarray(mt[sl]),
            "wqk": wqk, "bqk": bqk, "wv": wv,
            "wo": wo_pad,
        })
    return in_maps


def kernel(x, mask, Wq, bq, Wk, bk, Wv, bv, Wo, bo, **kw):
    nc = get_nc()
    in_maps = make_in_maps(x, mask, Wq, bq, Wk, bk, Wv, bv, Wo, bo)
    res = run_bass_kernel_spmd(nc, in_maps, core_ids=list(range(NCORES)))
    outs = [res.results[c]["out"] for c in range(NCORES)]
    full = np.concatenate(outs, axis=0)          # [PAIRS, S, H]
    return np.ascontiguousarray(full.reshape(B, T, S, H).astype(np.float32))
